# revision 43
# baseline (speedup 1.0000x reference)
"""Multi-head attention (B=8, N=1024, C=1024, H=16) on 8 TRN2 NeuronCores.

Strategy: pure data parallelism -- each core computes one batch element with
replicated weights (no collectives). All matmul operands are bf16 (PSUM
accumulation stays fp32), which halves HBM/SBUF traffic and keeps every
tensor resident so all phases interleave in one global schedule.

DMA reality on this part: the HWDGE queues' transfers serialize at
~176GB/s aggregate, so startup latency = bytes-before-first-use / 176GB/s.
Weights are therefore repacked HOST-SIDE by consumption unit:
  wkJ/wqJ[jb] [128c-blocked, 1024] -- one 256KB DMA = one feature block
             usable by B(jb) across all 8 contraction blocks
  wvV[vc]    [128, 4096]           -- one 1MB DMA = heads vc*8..vc*8+7
  x          16 half tiles [128, 512] (cb, nch)
The exp-critical set (x.nch0 + wkJ0 + wqJ0 = 1.5MB) lands ~15us in; the
exp stream starts ~18us and everything else streams just-in-time.

Emission order is BOTH program order (dataflow) and scheduler priority
(the per-engine queues are static and in-order).  Filler matmuls
(projections B, V-production A, out-proj D) are woven 2-4 matmuls at a
time between each attention unit's exp and PV so the exp stream on the
Scalar engine (143us busy, the second-longest resource after the tensor
engine) is never stalled by more than ~one matmul.

Attention per pair hp (heads 2hp, 2hp+1), ic-major (query chunks of 512):
  QK: two row-group-concurrent matmuls (contraction 64) -> S [128k, 1024]
  exp: one ACT op per unit, bf16 out, scale folded in
  PV: V_hat [128k, 65] (64 dims + ones col -> softmax sums in row 64)
  norm: copy out of PSUM; sums reshaped [128,4] for the reciprocal (DVE
        reciprocal costs ~6.5cyc/FREE-elem); gpsimd partition broadcast;
        multiply into A_sb (odd heads DMA-shifted to partitions 64+)
  D:  out[cb, nch] = sum_hb woT_hb.T @ A_hb + bias
"""
import numpy as np

B, N, C = 8, 1024, 1024
H = 16
HD = C // H               # 64
SCALE = HD ** (-0.5)
NCORES = 8

_COMPILED = {}


def _build():
    import concourse.bass as bass
    import concourse.tile as tile
    from concourse import bacc, mybir

    F32 = mybir.dt.float32
    BF16 = mybir.dt.bfloat16
    EXP = mybir.ActivationFunctionType.Exp

    nc = bacc.Bacc("TRN2", target_bir_lowering=False, debug=False)

    xT = nc.dram_tensor("xT", [C, N], BF16, kind="ExternalInput").ap()
    wqJ = nc.dram_tensor("wqJ", [C, C], BF16, kind="ExternalInput").ap()
    wkJ = nc.dram_tensor("wkJ", [C, C], BF16, kind="ExternalInput").ap()
    wvV = nc.dram_tensor("wvV", [2 * 128, 8 * 512], BF16, kind="ExternalInput").ap()
    woT = nc.dram_tensor("woT", [C, C], BF16, kind="ExternalInput").ap()
    bqk = nc.dram_tensor("bqk", [128, 16], F32, kind="ExternalInput").ap()
    bv = nc.dram_tensor("bv", [1, C], F32, kind="ExternalInput").ap()
    bo = nc.dram_tensor("bo", [128, 8], F32, kind="ExternalInput").ap()
    ones8 = nc.dram_tensor("ones8", [128, 8], BF16, kind="ExternalInput").ap()
    outT = nc.dram_tensor("outT", [C, N], F32, kind="ExternalOutput").ap()

    CB = C // 128      # 8 contraction blocks
    TB = N // 128      # 8 token/key blocks
    VW = 65            # per-head V width (64 dims + ones col)

    with tile.TileContext(nc) as tc:
        with tc.tile_pool(name="misc", bufs=1) as pool_misc, \
             tc.tile_pool(name="w", bufs=1) as pool_w, \
             tc.tile_pool(name="qk", bufs=1) as pool_qk, \
             tc.tile_pool(name="V", bufs=1) as pool_V, \
             tc.tile_pool(name="A", bufs=1) as pool_A, \
             tc.tile_pool(name="PT", bufs=16) as pool_PT, \
             tc.tile_pool(name="norm", bufs=2) as pool_norm, \
             tc.tile_pool(name="outp", bufs=3) as pool_out, \
             tc.tile_pool(name="ps_S", bufs=2, space="PSUM") as ps_S, \
             tc.tile_pool(name="ps_O", bufs=2, space="PSUM") as ps_O, \
             tc.tile_pool(name="ps_fill", bufs=2, space="PSUM") as ps_fill:

            # ---------------- static tiles ----------------
            # x half-tiles: index 2*cb+nch -> [128, 512]
            x2_sb = [pool_w.tile([128, 512], BF16, tag=f"x{i}", name=f"x{i}")
                     for i in range(2 * CB)]
            wkJ_sb = [pool_w.tile([128, C], BF16, tag=f"wk{j}", name=f"wk{j}") for j in range(8)]
            wqJ_sb = [pool_w.tile([128, C], BF16, tag=f"wq{j}", name=f"wq{j}") for j in range(8)]
            wvV_sb = [pool_w.tile([128, 8 * 512], BF16, tag=f"wv{v}", name=f"wv{v}") for v in range(2)]
            wo_sb = [pool_w.tile([128, C], BF16, tag=f"wo{cb}", name=f"wo{cb}") for cb in range(CB)]
            # qk_sb[0..7] = q feature blocks, qk_sb[8..15] = k feature blocks
            qk_sb = [pool_qk.tile([128, N], BF16, tag=f"qk{jb}", name=f"qk{jb}") for jb in range(16)]
            # V split by vc half so pair p's PV depends only on vc = p//4
            V2_sb = [[pool_V.tile([128, 8 * VW], BF16, tag=f"V{tb}_{vc}",
                                  name=f"V{tb}_{vc}") for vc in range(2)]
                     for tb in range(TB)]
            A_sb = [pool_A.tile([128, N], BF16, tag=f"A{hp}", name=f"A{hp}") for hp in range(CB)]

            bqk_sb = pool_misc.tile([128, 16], F32, tag="bqk")
            bv_sb = pool_misc.tile([1, C], F32, tag="bv")
            bv_rep = pool_misc.tile([128, C], F32, tag="bvrep")
            bo_sb = pool_misc.tile([128, 8], F32, tag="bo")

            # ---------------- DMA loads ----------------
            # The DMA pipe serializes across queues (~176 GB/s aggregate),
            # so every stream is ordered by true first-use and the scalar
            # queue carries ONLY the two packs the first exp needs.
            def J(j):
                nc.sync.dma_start(wkJ_sb[j][:, :], wkJ[j * 128:(j + 1) * 128, :])
                nc.sync.dma_start(wqJ_sb[j][:, :], wqJ[j * 128:(j + 1) * 128, :])
            for nch in range(2):
                for cb in range(CB):
                    nc.sync.dma_start(
                        x2_sb[2 * cb + nch][:, :],
                        xT[cb * 128:(cb + 1) * 128, nch * 512:(nch + 1) * 512])
            J(1)
            nc.sync.dma_start(wvV_sb[0][:, :], wvV[0:128, :])
            nc.sync.dma_start(wvV_sb[1][:, :], wvV[128:256, :])
            for j in range(2, 8):
                J(j)
            nc.scalar.dma_start(wkJ_sb[0][:, :], wkJ[0:128, :])
            nc.scalar.dma_start(wqJ_sb[0][:, :], wqJ[0:128, :])
            # gpsimd: biases + ones columns (small); wo is issued later
            nc.gpsimd.dma_start(bqk_sb[:, :], bqk)
            nc.gpsimd.dma_start(bv_sb[:, :], bv)
            nc.gpsimd.dma_start(bo_sb[:, :], bo)
            nc.gpsimd.partition_broadcast(bv_rep[:, :], bv_sb[0:1, :])
            for tb in range(TB):
                nc.gpsimd.dma_start(V2_sb[tb][0][:, 64::VW], ones8)

            # ---------------- emission helpers ----------------
            def B_fill_mm(jb, nch, cb, state):
                """One matmul of a qk feature chunk (weavable filler)."""
                w_sb = wqJ_sb[jb] if jb < 8 else wkJ_sb[jb - 8]
                if cb == 0:
                    state["ps"] = ps_fill.tile([128, 512], F32, tag="fill", name="fill")
                nc.tensor.matmul(
                    state["ps"][:, :],
                    w_sb[:, cb * 128:(cb + 1) * 128],
                    x2_sb[2 * cb + nch][:, :],
                    start=(cb == 0), stop=(cb == CB - 1),
                )
                if cb == CB - 1:
                    nc.vector.tensor_scalar(
                        qk_sb[jb][:, nch * 512:(nch + 1) * 512], state["ps"][:, :],
                        bqk_sb[:, jb:jb + 1], None, mybir.AluOpType.add,
                    )

            def B_fillers(jb):
                state0, state1 = {}, {}
                return [
                    (lambda nch=nch, cb=cb, st=(state0 if nch == 0 else state1):
                     B_fill_mm(jb, nch, cb, st))
                    for nch in range(2) for cb in range(CB)
                ]

            def emit_B(jb):
                for f in B_fillers(jb):
                    f()

            def A_fill_mm(tb, vc, cb, state):
                """One matmul of a V-production chunk (weavable filler)."""
                if cb == 0:
                    state["ps"] = ps_fill.tile([128, 512], F32, tag="fill", name="fill")
                nc.tensor.matmul(
                    state["ps"][:, :],
                    x2_sb[2 * cb + tb // 4][:, (tb % 4) * 128:(tb % 4 + 1) * 128],
                    wvV_sb[vc][:, cb * 512:(cb + 1) * 512],
                    start=(cb == 0), stop=(cb == CB - 1),
                )
                if cb == CB - 1:
                    # scatter 8 heads x 64 dims into the 65-strided layout,
                    # adding the broadcast v bias
                    dst3 = V2_sb[tb][vc][:, :].rearrange(
                        "p (h d) -> p h d", h=8)[:, :, 0:64]
                    src3 = state["ps"][:, :].rearrange("p (h d) -> p h d", h=8)
                    bv3 = bv_rep[:, vc * 512:(vc + 1) * 512].rearrange(
                        "p (h d) -> p h d", h=8)
                    nc.vector.tensor_add(dst3, src3, bv3)

            def A_fillers(tb, vc):
                state = {}
                return [(lambda cb=cb: A_fill_mm(tb, vc, cb, state))
                        for cb in range(CB)]

            def emit_A_chunk(tb, vc):
                for f in A_fillers(tb, vc):
                    f()

            def D_fill_mm(nch, cb, hb, state, pool=None):
                """One matmul of an out-projection chunk (weavable filler)."""
                if hb == 0:
                    # pool override lets the D1 tail alternate between the
                    # fill pool and the (idle by then) S pool, so four
                    # chunks are in flight and bias-add WAR never stalls it
                    p, tag = pool if pool is not None else (ps_fill, "fill")
                    state["ps"] = p.tile([128, 512], F32, tag=tag, name=tag)
                nc.tensor.matmul(
                    state["ps"][:, :],
                    wo_sb[hb][:, cb * 128:(cb + 1) * 128],
                    A_sb[hb][:, nch * 512:(nch + 1) * 512],
                    start=(hb == 0), stop=(hb == CB - 1),
                )
                if hb == CB - 1:
                    o_t = pool_out.tile([128, 512], F32, tag="ot", name="ot")
                    nc.vector.tensor_scalar(
                        o_t[:, :], state["ps"][:, :], bo_sb[:, cb:cb + 1], None,
                        mybir.AluOpType.add,
                    )
                    nc.sync.dma_start(
                        outT[cb * 128:(cb + 1) * 128,
                             nch * 512:(nch + 1) * 512], o_t[:, :])

            def D_fillers(nch, order=None, pools=None):
                states = [dict() for _ in range(CB)]
                order = order or [(cb, hb) for cb in range(CB) for hb in range(CB)]
                return [
                    (lambda nch=nch, cb=cb, hb=hb:
                     D_fill_mm(nch, cb, hb, states[cb],
                               pools[cb % len(pools)] if pools else None))
                    for cb, hb in order
                ]

            def emit_D(nch, order=None, pools=None):
                for f in D_fillers(nch, order, pools):
                    f()

            def emit_norm(hp, hh, ic, o_ps):
                """Normalize O (psum [65,512], row 64 = sums) into A_sb."""
                o_cp = pool_norm.tile([VW, 512], F32, tag="ocp", name="ocp")
                nc.vector.tensor_copy(o_cp[:, :], o_ps[:, :])  # frees bank
                s128 = pool_norm.tile([128, 4], F32, tag="s128", name="s128")
                nc.sync.dma_start(s128[:, :], o_cp[64:65, :])
                nc.vector.reciprocal(s128[:, :], s128[:, :])
                sums0 = pool_norm.tile([1, 512], F32, tag="sums0", name="sums0")
                nc.sync.dma_start(sums0[:, :], s128[:, :])
                r_rep = pool_norm.tile([64, 512], F32, tag="rrep", name="rrep")
                nc.gpsimd.partition_broadcast(r_rep[:, :], sums0[0:1, :])
                if hh == 0:
                    nc.vector.tensor_mul(
                        A_sb[hp][0:64, ic * 512:(ic + 1) * 512],
                        o_cp[0:64, :], r_rep[:, :])
                else:
                    a_tmp = pool_norm.tile([64, 512], BF16, tag="atmp",
                                           name="atmp")
                    nc.vector.tensor_mul(a_tmp[:, :], o_cp[0:64, :], r_rep[:, :])
                    nc.gpsimd.dma_start(
                        A_sb[hp][64:128, ic * 512:(ic + 1) * 512], a_tmp[:, :])

            def emit_pv_block(hp, ic, pts):
                """PV accumulation + norms for one query-chunk."""
                o_ps = {}
                for kb in range(TB):
                    p_t = pts[kb]
                    for hh in range(2):
                        h = 2 * hp + hh
                        if kb == 0:
                            o_ps[hh] = ps_O.tile([VW, 512], F32, tag="O",
                                                 name="O")
                        nc.tensor.matmul(
                            o_ps[hh][:, :],
                            V2_sb[kb][h // 8][:, (h % 8) * VW:(h % 8 + 1) * VW],
                            p_t[:, hh * 512:(hh + 1) * 512],
                            start=(kb == 0), stop=(kb == TB - 1),
                        )
                for hh in range(2):
                    emit_norm(hp, hh, ic, o_ps[hh])

            def emit_att_half(hp, ic, mid=None, fillers=None, per_unit=2,
                              defer_pv=False):
                """One query-chunk (512 cols) of attention pair hp.

                mid(kb)/fillers are woven between unit kb's exp and its PV
                matmuls: they cannot stall the exp stream there but still
                precede (program-order) everything consuming their output.
                defer_pv=True emits only QK+exp and returns the P tiles so
                the caller can place the PV block after late-arriving data
                (the per-engine queues execute strictly in order, so a
                data-blocked instruction stalls everything behind it).
                """
                fillers = list(fillers) if fillers else []
                o_ps = {}
                pts = {}
                for kb in range(TB):
                    s_ps = ps_S.tile([128, N], F32, tag="S", name="S")
                    for hh in range(2):
                        r0, r1 = hh * 64, hh * 64 + 64
                        nc.tensor.matmul(
                            s_ps[:, hh * 512:(hh + 1) * 512],
                            qk_sb[8 + hp][r0:r1, kb * 128:(kb + 1) * 128],
                            qk_sb[hp][r0:r1, ic * 512:(ic + 1) * 512],
                            start=True, stop=True,
                        )
                    p_t = pool_PT.tile([128, N], BF16, tag="pt", name="pt")
                    nc.scalar.activation(p_t[:, :], s_ps[:, :], EXP,
                                         scale=float(SCALE))
                    pts[kb] = p_t
                    if mid is not None:
                        mid(kb)
                    for _ in range(per_unit):
                        if fillers:
                            fillers.pop(0)()
                    if not defer_pv:
                        for hh in range(2):
                            h = 2 * hp + hh
                            if kb == 0:
                                o_ps[hh] = ps_O.tile([VW, 512], F32, tag="O",
                                                     name="O")
                            nc.tensor.matmul(
                                o_ps[hh][:, :],
                                V2_sb[kb][h // 8][:, (h % 8) * VW:(h % 8 + 1) * VW],
                                p_t[:, hh * 512:(hh + 1) * 512],
                                start=(kb == 0), stop=(kb == TB - 1),
                            )
                if not defer_pv:
                    for hh in range(2):
                        emit_norm(hp, hh, ic, o_ps[hh])
                for f in fillers:  # flush leftovers (low priority tail)
                    f()
                return pts

            # ---------------- global emission order ----------------
            # Warmup: ~4us of throwaway matmuls on the first-arriving x
            # chunk releases the PE HAM throttle (cold = 1.2GHz) before the
            # real DMA-paced startup matmuls run.
            s_warm = ps_S.tile([128, N], F32, tag="S", name="S")
            for i in range(18):
                nc.tensor.matmul(
                    s_warm[:, 0:512], x2_sb[0][:, 0:128], x2_sb[0][:, :],
                    start=(i == 0), stop=(i == 17))
            # pair0 k/q features from the nch0 halves only (x.nch0 + J0
            # land ~11us in); the nch1 halves are woven into pair0's units.
            st8, st0 = {}, {}
            for cb in range(CB):
                B_fill_mm(8, 0, cb, st8)
            for cb in range(CB):
                B_fill_mm(0, 0, cb, st0)
            # pair0: QK+exp for both halves first (PVs deferred past the
            # late-arriving wvV0), weaving the nch1 feature halves and
            # pair1's features; then V production, then the PV blocks.
            nch1 = []
            for jb in (8, 0):
                st = {}
                nch1 += [(lambda jb=jb, cb=cb, st=st: B_fill_mm(jb, 1, cb, st))
                         for cb in range(CB)]
            pts00 = emit_att_half(0, 0, fillers=nch1 + B_fillers(9)[:8],
                                  per_unit=3, defer_pv=True)
            pts01 = emit_att_half(0, 1, fillers=B_fillers(9)[8:] + B_fillers(1),
                                  per_unit=3, defer_pv=True)
            for tb in range(TB):
                emit_A_chunk(tb, 0)
            emit_pv_block(0, 0, pts00)
            emit_pv_block(0, 1, pts01)
            # A-vc1 (heads 8-15, needed from pair 4) spread over pairs 1-3
            a1 = [f for tb in range(TB) for f in A_fillers(tb, 1)]
            for hp in range(1, 7):
                if hp == 2:
                    # wo + vc1 ones: issued mid-kernel so they never compete
                    # with the startup streams
                    for cb in range(CB):
                        nc.gpsimd.dma_start(
                            wo_sb[cb][:, :], woT[cb * 128:(cb + 1) * 128, :])
                    for tb in range(TB):
                        nc.gpsimd.dma_start(V2_sb[tb][1][:, 64::VW], ones8)
                nxt = B_fillers(8 + hp + 1) + B_fillers(hp + 1)
                if hp <= 3:
                    take = 22 if hp < 3 else 20
                    mix, a1 = a1[:take], a1[take:]
                    fills = nxt + mix
                    emit_att_half(hp, 0, fillers=fills[:len(fills) // 2],
                                  per_unit=4)
                    emit_att_half(hp, 1, fillers=fills[len(fills) // 2:],
                                  per_unit=4)
                else:
                    emit_att_half(hp, 0, fillers=nxt[:16], per_unit=2)
                    emit_att_half(hp, 1, fillers=nxt[16:], per_unit=2)
            emit_att_half(7, 0)
            # out-proj nch0 fully woven into pair7.ic1 (8/unit); for nch1,
            # the first two chunks' hb0-6 matmuls (independent of pair7's
            # norms) bridge the norm-latency gap so HAM never re-throttles
            # before the out-proj tail.
            emit_att_half(7, 1, fillers=D_fillers(0), per_unit=8)
            bridge = [(cb, hb) for cb in range(4) for hb in range(CB - 1)]
            rest = [(cb, 7) for cb in range(4)] + \
                   [(cb, hb) for cb in range(4, CB) for hb in range(CB)]
            emit_D(1, order=bridge + rest,
                   pools=[(ps_fill, "fill"), (ps_S, "S")])

    nc.compile()
    return nc


def _get_nc():
    if "nc" not in _COMPILED:
        _COMPILED["nc"] = _build()
    return _COMPILED["nc"]


def _run(x, in_proj_weight, in_proj_bias, out_proj_weight, out_proj_bias,
         trace=False):
    import ml_dtypes
    from concourse.bass_utils import run_bass_kernel_spmd

    BF = ml_dtypes.bfloat16
    nc = _get_nc()
    x = np.asarray(x, dtype=np.float32)
    w_in = np.asarray(in_proj_weight, dtype=np.float32)
    b_in = np.asarray(in_proj_bias, dtype=np.float32)
    w_out = np.asarray(out_proj_weight, dtype=np.float32)
    b_out = np.asarray(out_proj_bias, dtype=np.float32)

    wqT = np.ascontiguousarray(w_in[0:C].T)              # [C, C]
    wkT = np.ascontiguousarray(w_in[C:2 * C].T)
    wvT = np.ascontiguousarray(w_in[2 * C:3 * C].T)
    # repack by consumption unit (see module docstring)
    wqJ = wqT.reshape(8, 128, 8, 128).transpose(2, 1, 0, 3).reshape(C, C)
    wkJ = wkT.reshape(8, 128, 8, 128).transpose(2, 1, 0, 3).reshape(C, C)
    wvV = wvT.reshape(8, 128, 2, 512).transpose(2, 1, 0, 3).reshape(256, 4096)

    shared = {
        "wqJ": np.ascontiguousarray(wqJ).astype(BF),
        "wkJ": np.ascontiguousarray(wkJ).astype(BF),
        "wvV": np.ascontiguousarray(wvV).astype(BF),
        "woT": np.ascontiguousarray(w_out.T).astype(BF),
        "bqk": np.ascontiguousarray(b_in[0:2 * C].reshape(16, 128).T),
        "bv": np.ascontiguousarray(b_in[2 * C:3 * C])[None, :],
        "bo": np.ascontiguousarray(b_out.reshape(8, 128).T),
        "ones8": np.ones((128, 8), dtype=BF),
    }
    in_maps = []
    for c in range(NCORES):
        m = dict(shared)
        m["xT"] = np.ascontiguousarray(x[c].T).astype(BF)
        in_maps.append(m)

    res = run_bass_kernel_spmd(nc, in_maps, core_ids=list(range(NCORES)),
                               trace=trace)
    out = np.stack([
        np.ascontiguousarray(res.results[c]["outT"].T) for c in range(NCORES)
    ]).astype(np.float32)
    return out, res


def kernel(x, in_proj_weight, in_proj_bias, out_proj_weight, out_proj_bias):
    out, _ = _run(x, in_proj_weight, in_proj_bias, out_proj_weight,
                  out_proj_bias)
    return out
